# revision 17
# baseline (speedup 1.0000x reference)
"""Chunked sigmoid MHA on 8 Trainium2 NeuronCores (Bass/Tile).

Problem: out = (sigmoid(scale * (x_q Wq^T)(x_k Wk^T)^T) @ (x_v Wv^T)) @ Wo^T
with B=2, L=S=2048, E=1024, H=16, D=64.

Sharding: (batch, head-group) — core c handles batch b=c//4 and heads
[4g, 4g+4) with g=c%4.  Each core computes its 4 heads' Q/K/V projections
(column slices of Wq/Wk/Wv), full sigmoid attention for those heads, and a
partial output projection (row slice of Wo^T); the host sums the 4 partial
outputs per batch.

Device layouts (per core):
  xqT/xkT/xvT [E=1024, L=2048]   host-transposed activations
  wq/wk/wv    [E=1024, 256]      (Wq[g*256:(g+1)*256, :]).T
  wo          [256, E=1024]      (Wo[:, g*256:(g+1)*256]).T
  out         [L=2048, E=1024]   bf16 partial output (host sums in f32)

Input staging: one coalesced dma_start per (tensor, L-chunk) with a
[128, 8, 512] strided access pattern (all 8 E-chunks at once), spread
across four engine DMA queues (sync=xk, scalar=xq, vector=xv,
gpsimd=weights).  Per-dma_start issue overhead (~0.6us) and queue
serialization made the old 100-small-DMA staging take ~55us; this lands
chunk 0 by ~10us and everything by ~40us, which the just-in-time
projection filler schedule tolerates.

All matmuls run bf16.  Scores matmuls are row-tiled (K=64: two heads
packed in array rows 0-63 / 64-127); attention-output matmuls are
col-tiled (M=64: two heads packed in array cols 0-63 / 64-127 -> PSUM
partition halves).
"""

import ml_dtypes
import numpy as np

import concourse.bass as bass
import concourse.mybir as mybir
import concourse.tile as tile
from concourse import bass_utils
from concourse.vector_clock import ScopedClock

F32 = mybir.dt.float32
F32R = mybir.dt.float32r
BF16 = mybir.dt.bfloat16
AF = mybir.ActivationFunctionType

E = 1024          # embed dim
L = 2048          # sequence length (queries == keys)
DH = 256          # per-core projection dim (4 heads x 64)
EC = E // 128     # 8 E-chunks of 128
LC = L // 512     # 4 L-chunks of 512
ST = L // 128     # 16 S-tiles of 128
SCALE = 64 ** -0.5  # 0.125, applied inside the sigmoid activation

N_CORES = 8


class SplitDrainTileContext(tile.TileContext):
    """This walrus build rejects >1 sync wait on the SP CTRL (Drain)
    instruction, and Tile's end-of-kernel drain waits on every used proc.
    Split the waits across a chain of single-wait drains."""

    DRAIN_WAIT_CAP = 1

    def _drain_and_barrier(self, tick_clock, wait_clock):
        nc = self.nc
        drain_inst = nc.sync.drain()
        wait_clock.add_sem_waits(
            drain_inst.ins, ScopedClock({None: tick_clock.global_clock})
        )
        si = drain_inst.ins.sync_info
        waits = list(si.on_wait) if si is not None else []
        if len(waits) > self.DRAIN_WAIT_CAP:
            si.on_wait = waits[: self.DRAIN_WAIT_CAP]
            for i in range(self.DRAIN_WAIT_CAP, len(waits), self.DRAIN_WAIT_CAP):
                extra = nc.sync.drain()
                esi = extra.ins.sync_info
                if esi is None:
                    esi = mybir.SyncInfo(on_wait=[], on_update=[])
                esi.on_wait = waits[i : i + self.DRAIN_WAIT_CAP]
                extra.ins.sync_info = esi
        nc.all_engine_barrier()
        assert self.sems is not None
        popped = nc._tile_sem_poison_stack.pop()
        assert popped is self._sem_poison
        nc.clear_and_free_semaphores(list(self.sems.allocated().values()))
        nc.all_engine_barrier()


def build_nc() -> bass.Bass:
    nc = bass.Bass("TRN2", target_bir_lowering=False, debug=False)

    xq = nc.dram_tensor("xqT", [E, L], BF16, kind="ExternalInput").ap()
    xk = nc.dram_tensor("xkT", [E, L], BF16, kind="ExternalInput").ap()
    xv = nc.dram_tensor("xvT", [E, L], BF16, kind="ExternalInput").ap()
    wq = nc.dram_tensor("wq", [E, DH], BF16, kind="ExternalInput").ap()
    wk = nc.dram_tensor("wk", [E, DH], BF16, kind="ExternalInput").ap()
    wv = nc.dram_tensor("wv", [E, DH], BF16, kind="ExternalInput").ap()
    wo = nc.dram_tensor("wo", [DH, E], BF16, kind="ExternalInput").ap()
    # transposed output [E, L]: the output projection runs with the
    # (early-loaded) wo as the PE stationary operand and the
    # freshly-computed oT as the moving operand, avoiding LDWEIGHTS
    # stalls on the just-cast oT tiles.  The host transposes back.
    out = nc.dram_tensor("out", [E, L], BF16, kind="ExternalOutput").ap()

    with SplitDrainTileContext(nc) as tc:
        body(tc, xq, xk, xv, wq, wk, wv, wo, out)
    _split_waits(nc)
    return nc


def _split_waits(nc, cap=1):
    """This walrus build rejects instructions carrying more than one sync
    wait.  Hoist excess waits onto same-engine NoOps inserted immediately
    before the instruction (engine program order enforces them first)."""
    ctr = 0
    for f in nc.m.functions:
        for bb in f.blocks:
            new = []
            for inst in bb.instructions:
                si = inst.sync_info
                waits = list(si.on_wait) if si is not None else []
                if len(waits) > cap:
                    for i in range(cap, len(waits), cap):
                        ctr += 1
                        nop = mybir.InstNoOp(name=f"I-waitnop-{ctr}")
                        nop.engine = inst.engine
                        nop.sync_info = mybir.SyncInfo(
                            on_wait=waits[i : i + cap], on_update=[]
                        )
                        nc.register_instruction(nop)
                        new.append(nop)
                    si.on_wait = waits[:cap]
                new.append(inst)
            bb.instructions = new
    return ctr


def body(tc, xq, xk, xv, wq, wk, wv, wo, out):
    nc = tc.nc

    # ---- persistent SBUF tensors -------------------------------------
    persist = tc.alloc_tile_pool(name="persist", bufs=1)

    def ptile(name, shape):
        return persist.tile(shape, BF16, tag=name, name=name)

    # weights, E-chunk-major: w*_sb[:, e*256+m] = w*T[e*128+p, m]
    wq_sb = ptile("wq_sb", [128, EC * DH])
    wk_sb = ptile("wk_sb", [128, EC * DH])
    wv_sb = ptile("wv_sb", [128, EC * DH])
    # wo, m-chunk-major: wo_sb[:, m*1024+e] = wo[m*128+p, e]
    wo_sb = ptile("wo_sb", [128, 2 * E])
    # projected tensors: qT/kT [dh, L] stored Mt-major; v natural [S, dh]
    # stored St-major; oT [dh, L] stored m-chunk-major
    qT_sb = ptile("qT_sb", [128, 2 * L])
    kT_sb = ptile("kT_sb", [128, 2 * L])
    v_sb = persist.tile([128, ST * DH], BF16, tag="v_sb", name="v_sb")
    oT_sb = ptile("oT_sb", [128, 2 * L])

    sc_pool = tc.alloc_tile_pool(name="sc", bufs=8)
    ou_pool = tc.alloc_tile_pool(name="ou", bufs=3)
    xpool = tc.alloc_tile_pool(name="xpool", bufs=3 * LC)
    ps_proj = tc.alloc_tile_pool(name="ps_proj", bufs=2, space="PSUM")
    ps_sc = tc.alloc_tile_pool(name="ps_sc", bufs=2, space="PSUM")
    ps_o = tc.alloc_tile_pool(name="ps_o", bufs=2, space="PSUM")

    # ---- coalesced input staging -------------------------------------
    # x tiles: one [128, EC*512] tile per (tensor, L-chunk); the DMA
    # gathers all 8 E-chunks in one strided dma_start.  Queue assignment:
    # sync=xk, scalar=xq, vector=xv (each also carries its weight first),
    # gpsimd=wo (SWDGE, needed late).  Chunk 0 of k/q is split in two
    # dma_starts so the first projection matmuls can start earlier.
    xtiles = {}

    def dma_startup():
        w3 = {n: w.rearrange("(e p) m -> p e m", p=128) for n, w in
              (("k", wk), ("q", wq), ("v", wv))}
        x3 = {n: x.rearrange("(e p) l -> p e l", p=128) for n, x in
              (("k", xk), ("q", xq), ("v", xv))}

        def xtile(nm, c):
            xt = xpool.tile([128, EC * 512], BF16, tag="x", name=f"x{nm}{c}")
            xtiles[(nm, c)] = xt
            return (xt[:].rearrange("p (e l) -> p e l", l=512),
                    x3[nm][:, :, c * 512 : (c + 1) * 512])

        # Strict priority on the two fast HWDGE queues: critical chunk-0
        # bytes + weights first, later chunks behind them.  The 16 DMA
        # engines are shared, so any concurrently-queued non-critical
        # transfer steals bandwidth from the startup-critical prefix.
        # gpsimd (slow SWDGE) carries only wo, which is needed last.
        nc.sync.dma_start(
            wk_sb[:].rearrange("p (e m) -> p e m", m=DH), w3["k"])
        dst, src = xtile("k", 0)
        nc.sync.dma_start(dst[:, 0:4, :], src[:, 0:4, :])
        nc.sync.dma_start(dst[:, 4:8, :], src[:, 4:8, :])
        nc.scalar.dma_start(
            wq_sb[:].rearrange("p (e m) -> p e m", m=DH), w3["q"])
        dst, src = xtile("q", 0)
        nc.scalar.dma_start(dst[:, 0:4, :], src[:, 0:4, :])
        nc.scalar.dma_start(dst[:, 4:8, :], src[:, 4:8, :])
        nc.gpsimd.dma_start(
            wv_sb[:].rearrange("p (e m) -> p e m", m=DH), w3["v"])
        dst, src = xtile("v", 0)
        nc.gpsimd.dma_start(dst[:, 0:4, :], src[:, 0:4, :])
        nc.gpsimd.dma_start(dst[:, 4:8, :], src[:, 4:8, :])
        nc.sync.dma_start(*xtile("k", 1))
        nc.scalar.dma_start(*xtile("q", 1))
        nc.scalar.dma_start(*xtile("v", 1))
        nc.sync.dma_start(*xtile("k", 2))
        nc.scalar.dma_start(*xtile("q", 2))
        nc.sync.dma_start(*xtile("k", 3))
        nc.scalar.dma_start(*xtile("q", 3))
        nc.sync.dma_start(*xtile("v", 2))
        nc.scalar.dma_start(*xtile("v", 3))
        nc.gpsimd.dma_start(
            wo_sb[:].rearrange("p (m e) -> p m e", e=E),
            wo.rearrange("(m p) e -> p m e", p=128))

    def emit_proj_kq(c):
        """Yield closures emitting one L-chunk of the k/q projection
        matmuls (x tiles must already be staged via dma_startup)."""

        def kq_mms(nm, wsb, dst, e, acc):
            for mt in range(2):
                nc.tensor.matmul(
                    acc[mt][:],
                    lhsT=wsb[:, e * DH + mt * 128 : e * DH + (mt + 1) * 128],
                    rhs=xtiles[(nm, c)][:, e * 512 : (e + 1) * 512],
                    start=(e == 0),
                    stop=(e == EC - 1),
                )
            if e == EC - 1:
                for mt in range(2):
                    nc.vector.tensor_copy(
                        dst[:, mt * L + c * 512 : mt * L + (c + 1) * 512], acc[mt][:]
                    )

        for nm, wsb, dst in (("k", wk_sb, kT_sb), ("q", wq_sb, qT_sb)):
            acc = [
                ps_proj.tile([128, 512], F32, tag="ps_proj", name=f"{nm}{c}_{mt}")
                for mt in range(2)
            ]
            for e in range(EC):
                yield lambda nm=nm, wsb=wsb, dst=dst, e=e, acc=acc: kq_mms(nm, wsb, dst, e, acc)

    def emit_proj_v(c):
        def v_mms(st4, eh, box):
            st = c * 4 + st4
            if eh == 0:
                box["vacc"] = ps_proj.tile([128, DH], F32, tag="ps_proj", name=f"vacc{st}")
            for e in range(eh * 4, eh * 4 + 4):
                nc.tensor.matmul(
                    box["vacc"][:],
                    lhsT=xtiles[("v", c)][:, e * 512 + st4 * 128 : e * 512 + (st4 + 1) * 128],
                    rhs=wv_sb[:, e * DH : (e + 1) * DH],
                    start=(e == 0),
                    stop=(e == EC - 1),
                )
            if eh == 1:
                nc.vector.tensor_copy(v_sb[:, st * DH : (st + 1) * DH], box["vacc"][:])

        for st4 in range(4):
            box = {}
            for eh in range(2):
                yield lambda st4=st4, eh=eh, box=box: v_mms(st4, eh, box)

    out3 = out.rearrange("(b p) l -> p b l", p=128)

    def emit_outproj(lc):
        for ebp in range(4):
            def unit(ebp=ebp):
                ot = ou_pool.tile([128, E], BF16, tag="ou", name=f"ot{lc}_{ebp}")
                for sub in range(2):
                    eb = ebp * 2 + sub
                    ps = ps_proj.tile(
                        [128, 512], F32, tag="ps_proj", name=f"ops{lc}_{eb}"
                    )
                    for m in range(2):
                        nc.tensor.matmul(
                            ps[:],
                            lhsT=wo_sb[:, m * E + eb * 128 : m * E + (eb + 1) * 128],
                            rhs=oT_sb[:, m * L + lc * 512 : m * L + (lc + 1) * 512],
                            start=(m == 0),
                            stop=(m == 1),
                        )
                    nc.vector.tensor_copy(ot[:, sub * 512 : (sub + 1) * 512], ps[:])
                if lc == LC - 1:
                    # tail: halve per-queue transfer time by splitting
                    # each store across both DMA queues
                    for sub in range(2):
                        eb = ebp * 2 + sub
                        eng = (nc.sync, nc.gpsimd)[sub]
                        eng.dma_start(
                            out[eb * 128 : (eb + 1) * 128, lc * 512 : (lc + 1) * 512],
                            ot[:, sub * 512 : (sub + 1) * 512],
                        )
                else:
                    eng = (nc.sync, nc.gpsimd)[(lc * 4 + ebp) % 2]
                    eng.dma_start(
                        out3[:, ebp * 2 : (ebp + 1) * 2, lc * 512 : (lc + 1) * 512],
                        ot[:].rearrange("p (b l) -> p b l", l=512),
                    )
            yield unit

    # ---- main pipeline ------------------------------------------------
    # Attention is software-pipelined: the scores matmuls for step n+1
    # are emitted BEFORE the attention-output matmuls of step n, so the
    # in-order PE never sits between two consecutive ACT activations on
    # the critical path.  ACT then runs back-to-back (~2us/step), which
    # is the steady-state limit for lc >= 1.
    def scores_step(lc, st):
        sc_tiles = {}
        for pair in range(2):
            ps = ps_sc.tile([128, 1024], F32, tag="ps_sc", name=f"scps{lc}_{st}_{pair}")
            for sub in range(2):
                nc.tensor.matmul(
                    ps[:, sub * 512 : (sub + 1) * 512],
                    lhsT=kT_sb[
                        sub * 64 : (sub + 1) * 64,
                        pair * L + st * 128 : pair * L + (st + 1) * 128,
                    ],
                    rhs=qT_sb[
                        sub * 64 : (sub + 1) * 64,
                        pair * L + lc * 512 : pair * L + (lc + 1) * 512,
                    ],
                    start=True,
                    stop=True,
                    tile_position=(sub * 64, 0),
                )
            sc = sc_pool.tile([128, 1024], BF16, tag="sc", name=f"sc{lc}_{st}_{pair}")
            nc.scalar.activation(sc[:], ps[:], AF.Sigmoid, scale=SCALE)
            sc_tiles[pair] = sc
        return sc_tiles

    def attnout_step(lc, st, sc_tiles, o_acc):
        for pair in range(2):
            for sub in range(2):
                h = pair * 2 + sub
                nc.tensor.matmul(
                    o_acc[pair][sub * 64 : (sub + 1) * 64, :],
                    lhsT=v_sb[:, st * DH + h * 64 : st * DH + (h + 1) * 64],
                    rhs=sc_tiles[pair][:, sub * 512 : (sub + 1) * 512],
                    start=(st == 0),
                    stop=(st == ST - 1),
                    tile_position=(0, sub * 64),
                    # Sim's psum-group bookkeeping mis-addresses
                    # partition-offset groups; has_written is per-element
                    # on HW and the two halves are disjoint.
                    skip_group_check=True,
                )

    filler = []          # queue of pending closures

    def pop_filler(n):
        for _ in range(min(n, len(filler))):
            filler.pop(0)()

    def flush_filler():
        while filler:
            filler.pop(0)()

    # Warm the PE (HAM clock gate) with scratch matmuls while the first
    # DMAs are in flight; they have no data dependencies.  Also preload
    # the sigmoid ACT table so the first real activation skips the
    # ~1.3us table load.
    scratch = persist.tile([128, 512], BF16, tag="scratch", name="scratch")
    nc.gpsimd.memset(scratch[:], 0.0)
    warm_in = persist.tile([128, 8], BF16, tag="warm_in", name="warm_in")
    warm_out = persist.tile([128, 8], BF16, tag="warm_out", name="warm_out")
    nc.scalar.memzero(warm_in[:])
    nc.scalar.activation(warm_out[:], warm_in[:], AF.Sigmoid, scale=SCALE)
    wu_ps = ps_sc.tile([128, 1024], F32, tag="ps_sc", name="warmup_ps")
    for i in range(10):
        nc.tensor.matmul(
            wu_ps[:, :512], lhsT=scratch[:, :128], rhs=scratch[:],
            start=(i == 0), stop=(i == 9),
        )

    # stage all inputs; run chunk 0's k/q projections now so the first
    # scores matmuls reach the PE as early as possible.  Chunk 0's v
    # projection leads the filler: the first attn-out consumes it only
    # ~2 sigmoids after the first scores, hiding the (slow, SWDGE) xv
    # chunk-0 arrival behind scores/filler work.
    dma_startup()
    for u in emit_proj_kq(0):
        u()
    filler.extend(emit_proj_v(0))

    steps = [(lc, st) for lc in range(LC) for st in range(ST)]
    o_cur = None
    pend_sc = None
    for i, (lc, st) in enumerate(steps):
        if lc == 0 and st % 4 == 0 and st < 12:
            filler.extend(emit_proj_kq(st // 4 + 1))
            filler.extend(emit_proj_v(st // 4 + 1))
        if st == 0:
            o_cur = [
                ps_o.tile([128, 512], F32, tag="ps_o", name=f"oacc{lc}_{p}")
                for p in range(2)
            ]
        if pend_sc is None:
            pend_sc = scores_step(lc, st)
        # in lc0 the next sg's scores need that chunk's kT, which the
        # filler below is still producing -- don't look ahead across the
        # sg barrier there
        sg_barrier = lc == 0 and st % 4 == 3
        lookahead = None
        if i + 1 < len(steps) and not sg_barrier:
            lookahead = scores_step(*steps[i + 1])
        # filler BEFORE attn-out: attn-out feeds nothing downstream for
        # ~2us, while the filler may carry this step's v projection
        pop_filler(3 if (lc == 0 and st < 4) else (11 if lc == 0 else 1))
        attnout_step(lc, st, pend_sc, o_cur)
        pend_sc = lookahead
        if sg_barrier:
            flush_filler()
        if st == ST - 1:
            for pair in range(2):
                nc.vector.tensor_copy(
                    oT_sb[:, pair * L + lc * 512 : pair * L + (lc + 1) * 512],
                    o_cur[pair][:],
                )
            filler.extend(emit_outproj(lc))
            if lc == LC - 1:
                flush_filler()

    # release pools in reverse allocation (stack) order
    for pool in (ps_o, ps_sc, ps_proj, xpool, ou_pool, sc_pool, persist):
        pool.release()


_NC_CACHE = None


def _get_nc():
    global _NC_CACHE
    if _NC_CACHE is None:
        _NC_CACHE = build_nc()
    return _NC_CACHE


def _prep_in_maps(query, key, value, Wq, Wk, Wv, Wo):
    B = query.shape[0]
    xT = {}
    for b in range(B):
        xT[("q", b)] = np.ascontiguousarray(query[b].T).astype(ml_dtypes.bfloat16)
        xT[("k", b)] = np.ascontiguousarray(key[b].T).astype(ml_dtypes.bfloat16)
        xT[("v", b)] = np.ascontiguousarray(value[b].T).astype(ml_dtypes.bfloat16)
    in_maps = []
    for c in range(N_CORES):
        b, g = c // 4, c % 4
        hs = slice(g * DH, (g + 1) * DH)
        in_maps.append(
            {
                "xqT": xT[("q", b)],
                "xkT": xT[("k", b)],
                "xvT": xT[("v", b)],
                "wq": np.ascontiguousarray(Wq[hs, :].T).astype(ml_dtypes.bfloat16),
                "wk": np.ascontiguousarray(Wk[hs, :].T).astype(ml_dtypes.bfloat16),
                "wv": np.ascontiguousarray(Wv[hs, :].T).astype(ml_dtypes.bfloat16),
                "wo": np.ascontiguousarray(Wo[:, hs].T).astype(ml_dtypes.bfloat16),
            }
        )
    return in_maps


LAST_RESULTS = None


def run_sharded(query, key, value, Wq, Wk, Wv, Wo, trace=False, tmpdir=None):
    global LAST_RESULTS
    if trace:
        # Shim the missing antenv.axon_hooks so NTFF tracing works under axon.
        import sys
        import types

        try:
            import antenv.axon_hooks  # noqa: F401
        except ImportError:
            from trn_agent_boot.trn_boot import _ntff_profile_via_ctypes

            _mod = types.ModuleType("antenv.axon_hooks")
            _hook = _ntff_profile_via_ctypes("/opt/axon/libaxon_pjrt.so")
            _mod.get_axon_ntff_profile_hook = lambda: _hook
            sys.modules["antenv.axon_hooks"] = _mod
        bass_utils.upload_artifacts = lambda tmpdir: tmpdir

    nc = _get_nc()
    in_maps = _prep_in_maps(query, key, value, Wq, Wk, Wv, Wo)
    res = bass_utils.run_bass_kernel_spmd(
        nc, in_maps, core_ids=list(range(N_CORES)), trace=trace, tmpdir=tmpdir
    )
    LAST_RESULTS = res
    B = query.shape[0]
    full = np.zeros((B, L, E), dtype=np.float32)
    for c in range(N_CORES):
        full[c // 4] += res.results[c]["out"].astype(np.float32).T
    return full


def kernel(query, key, value, Wq, Wk, Wv, Wo):
    return run_sharded(query, key, value, Wq, Wk, Wv, Wo, trace=False)


# revision 20
# speedup vs baseline: 1.0302x; 1.0302x over previous
"""Chunked sigmoid MHA on 8 Trainium2 NeuronCores (Bass/Tile).

Problem: out = (sigmoid(scale * (x_q Wq^T)(x_k Wk^T)^T) @ (x_v Wv^T)) @ Wo^T
with B=2, L=S=2048, E=1024, H=16, D=64.

Sharding: (batch, head-group) — core c handles batch b=c//4 and heads
[4g, 4g+4) with g=c%4.  Each core computes its 4 heads' Q/K/V projections
(column slices of Wq/Wk/Wv), full sigmoid attention for those heads, and a
partial output projection (row slice of Wo^T); the host sums the 4 partial
outputs per batch.

Device layouts (per core):
  xqT/xkT/xvT [E=1024, L=2048]   host-transposed activations
  wq/wk/wv    [E=1024, 256]      (Wq[g*256:(g+1)*256, :]).T
  wo          [256, E=1024]      (Wo[:, g*256:(g+1)*256]).T
  out         [L=2048, E=1024]   bf16 partial output (host sums in f32)

Input staging: one coalesced dma_start per (tensor, L-chunk) with a
[128, 8, 512] strided access pattern (all 8 E-chunks at once), spread
across four engine DMA queues (sync=xk, scalar=xq, vector=xv,
gpsimd=weights).  Per-dma_start issue overhead (~0.6us) and queue
serialization made the old 100-small-DMA staging take ~55us; this lands
chunk 0 by ~10us and everything by ~40us, which the just-in-time
projection filler schedule tolerates.

All matmuls run bf16.  Scores matmuls are row-tiled (K=64: two heads
packed in array rows 0-63 / 64-127); attention-output matmuls are
col-tiled (M=64: two heads packed in array cols 0-63 / 64-127 -> PSUM
partition halves).
"""

import ml_dtypes
import numpy as np

import concourse.bass as bass
import concourse.mybir as mybir
import concourse.tile as tile
from concourse import bass_utils
from concourse.vector_clock import ScopedClock

F32 = mybir.dt.float32
F32R = mybir.dt.float32r
BF16 = mybir.dt.bfloat16
AF = mybir.ActivationFunctionType

E = 1024          # embed dim
L = 2048          # sequence length (queries == keys)
DH = 256          # per-core projection dim (4 heads x 64)
EC = E // 128     # 8 E-chunks of 128
LC = L // 512     # 4 L-chunks of 512
ST = L // 128     # 16 S-tiles of 128
SCALE = 64 ** -0.5  # 0.125, applied inside the sigmoid activation

N_CORES = 8


class SplitDrainTileContext(tile.TileContext):
    """This walrus build rejects >1 sync wait on the SP CTRL (Drain)
    instruction, and Tile's end-of-kernel drain waits on every used proc.
    Split the waits across a chain of single-wait drains."""

    DRAIN_WAIT_CAP = 1

    def _drain_and_barrier(self, tick_clock, wait_clock):
        nc = self.nc
        drain_inst = nc.sync.drain()
        wait_clock.add_sem_waits(
            drain_inst.ins, ScopedClock({None: tick_clock.global_clock})
        )
        si = drain_inst.ins.sync_info
        waits = list(si.on_wait) if si is not None else []
        if len(waits) > self.DRAIN_WAIT_CAP:
            si.on_wait = waits[: self.DRAIN_WAIT_CAP]
            for i in range(self.DRAIN_WAIT_CAP, len(waits), self.DRAIN_WAIT_CAP):
                extra = nc.sync.drain()
                esi = extra.ins.sync_info
                if esi is None:
                    esi = mybir.SyncInfo(on_wait=[], on_update=[])
                esi.on_wait = waits[i : i + self.DRAIN_WAIT_CAP]
                extra.ins.sync_info = esi
        nc.all_engine_barrier()
        assert self.sems is not None
        popped = nc._tile_sem_poison_stack.pop()
        assert popped is self._sem_poison
        nc.clear_and_free_semaphores(list(self.sems.allocated().values()))
        nc.all_engine_barrier()


def build_nc() -> bass.Bass:
    nc = bass.Bass("TRN2", target_bir_lowering=False, debug=False)

    xq = nc.dram_tensor("xqT", [E, L], BF16, kind="ExternalInput").ap()
    xk = nc.dram_tensor("xkT", [E, L], BF16, kind="ExternalInput").ap()
    xv = nc.dram_tensor("xvT", [E, L], BF16, kind="ExternalInput").ap()
    wq = nc.dram_tensor("wq", [E, DH], BF16, kind="ExternalInput").ap()
    wk = nc.dram_tensor("wk", [E, DH], BF16, kind="ExternalInput").ap()
    wv = nc.dram_tensor("wv", [E, DH], BF16, kind="ExternalInput").ap()
    wo = nc.dram_tensor("wo", [DH, E], BF16, kind="ExternalInput").ap()
    # transposed output [E, L]: the output projection runs with the
    # (early-loaded) wo as the PE stationary operand and the
    # freshly-computed oT as the moving operand, avoiding LDWEIGHTS
    # stalls on the just-cast oT tiles.  The host transposes back.
    out = nc.dram_tensor("out", [E, L], BF16, kind="ExternalOutput").ap()

    with SplitDrainTileContext(nc) as tc:
        body(tc, xq, xk, xv, wq, wk, wv, wo, out)
    _split_waits(nc)
    return nc


def _split_waits(nc, cap=1):
    """This walrus build rejects instructions carrying more than one sync
    wait.  Hoist excess waits onto same-engine NoOps inserted immediately
    before the instruction (engine program order enforces them first)."""
    ctr = 0
    for f in nc.m.functions:
        for bb in f.blocks:
            new = []
            for inst in bb.instructions:
                si = inst.sync_info
                waits = list(si.on_wait) if si is not None else []
                if len(waits) > cap:
                    for i in range(cap, len(waits), cap):
                        ctr += 1
                        nop = mybir.InstNoOp(name=f"I-waitnop-{ctr}")
                        nop.engine = inst.engine
                        nop.sync_info = mybir.SyncInfo(
                            on_wait=waits[i : i + cap], on_update=[]
                        )
                        nc.register_instruction(nop)
                        new.append(nop)
                    si.on_wait = waits[:cap]
                new.append(inst)
            bb.instructions = new
    return ctr


def body(tc, xq, xk, xv, wq, wk, wv, wo, out):
    nc = tc.nc

    # ---- persistent SBUF tensors -------------------------------------
    persist = tc.alloc_tile_pool(name="persist", bufs=1)

    def ptile(name, shape):
        return persist.tile(shape, BF16, tag=name, name=name)

    # weights, E-chunk-major: w*_sb[:, e*256+m] = w*T[e*128+p, m]
    wq_sb = ptile("wq_sb", [128, EC * DH])
    wk_sb = ptile("wk_sb", [128, EC * DH])
    wv_sb = ptile("wv_sb", [128, EC * DH])
    # wo, m-chunk-major: wo_sb[:, m*1024+e] = wo[m*128+p, e]
    wo_sb = ptile("wo_sb", [128, 2 * E])
    # projected tensors: qT/kT [dh, L] stored Mt-major; v natural [S, dh]
    # stored St-major; oT [dh, L] stored m-chunk-major
    qT_sb = ptile("qT_sb", [128, 2 * L])
    kT_sb = ptile("kT_sb", [128, 2 * L])
    v_sb = persist.tile([128, ST * DH], BF16, tag="v_sb", name="v_sb")
    oT_sb = ptile("oT_sb", [128, 2 * L])

    sc_pool = tc.alloc_tile_pool(name="sc", bufs=8)
    ou_pool = tc.alloc_tile_pool(name="ou", bufs=3)
    xpool = tc.alloc_tile_pool(name="xpool", bufs=3 * LC)
    ps_proj = tc.alloc_tile_pool(name="ps_proj", bufs=2, space="PSUM")
    ps_sc = tc.alloc_tile_pool(name="ps_sc", bufs=2, space="PSUM")
    ps_o = tc.alloc_tile_pool(name="ps_o", bufs=2, space="PSUM")

    # ---- coalesced input staging -------------------------------------
    # x tiles: one [128, EC*512] tile per (tensor, L-chunk); the DMA
    # gathers all 8 E-chunks in one strided dma_start.  Queue assignment:
    # sync=xk, scalar=xq, vector=xv (each also carries its weight first),
    # gpsimd=wo (SWDGE, needed late).  Chunk 0 of k/q is split in two
    # dma_starts so the first projection matmuls can start earlier.
    xtiles = {}

    def dma_startup():
        w3 = {n: w.rearrange("(e p) m -> p e m", p=128) for n, w in
              (("k", wk), ("q", wq), ("v", wv))}
        x3 = {n: x.rearrange("(e p) l -> p e l", p=128) for n, x in
              (("k", xk), ("q", xq), ("v", xv))}

        def xtile(nm, c):
            xt = xpool.tile([128, EC * 512], BF16, tag="x", name=f"x{nm}{c}")
            xtiles[(nm, c)] = xt
            return (xt[:].rearrange("p (e l) -> p e l", l=512),
                    x3[nm][:, :, c * 512 : (c + 1) * 512])

        # Strict priority on the two fast HWDGE queues: critical chunk-0
        # bytes + weights first, later chunks behind them.  The 16 DMA
        # engines are shared, so any concurrently-queued non-critical
        # transfer steals bandwidth from the startup-critical prefix.
        # gpsimd (slow SWDGE) carries only wo, which is needed last.
        nc.sync.dma_start(
            wk_sb[:].rearrange("p (e m) -> p e m", m=DH), w3["k"])
        dst, src = xtile("k", 0)
        nc.sync.dma_start(dst[:, 0:4, :], src[:, 0:4, :])
        nc.sync.dma_start(dst[:, 4:8, :], src[:, 4:8, :])
        dst, src = xtile("v", 0)
        nc.sync.dma_start(dst[:, 0:4, :], src[:, 0:4, :])
        nc.sync.dma_start(dst[:, 4:8, :], src[:, 4:8, :])
        nc.scalar.dma_start(
            wq_sb[:].rearrange("p (e m) -> p e m", m=DH), w3["q"])
        dst, src = xtile("q", 0)
        nc.scalar.dma_start(dst[:, 0:4, :], src[:, 0:4, :])
        nc.scalar.dma_start(dst[:, 4:8, :], src[:, 4:8, :])
        nc.scalar.dma_start(
            wv_sb[:].rearrange("p (e m) -> p e m", m=DH), w3["v"])
        nc.sync.dma_start(*xtile("k", 1))
        nc.scalar.dma_start(*xtile("q", 1))
        nc.scalar.dma_start(*xtile("v", 1))
        nc.sync.dma_start(*xtile("k", 2))
        nc.scalar.dma_start(*xtile("q", 2))
        nc.sync.dma_start(*xtile("k", 3))
        nc.scalar.dma_start(*xtile("q", 3))
        nc.sync.dma_start(*xtile("v", 2))
        nc.scalar.dma_start(*xtile("v", 3))
        nc.gpsimd.dma_start(
            wo_sb[:].rearrange("p (m e) -> p m e", e=E),
            wo.rearrange("(m p) e -> p m e", p=128))

    def emit_proj_kq(c):
        """Yield closures emitting one L-chunk of the k/q projection
        matmuls (x tiles must already be staged via dma_startup)."""

        def kq_mms(nm, wsb, dst, e, acc):
            for mt in range(2):
                nc.tensor.matmul(
                    acc[mt][:],
                    lhsT=wsb[:, e * DH + mt * 128 : e * DH + (mt + 1) * 128],
                    rhs=xtiles[(nm, c)][:, e * 512 : (e + 1) * 512],
                    start=(e == 0),
                    stop=(e == EC - 1),
                )
            if e == EC - 1:
                for mt in range(2):
                    nc.vector.tensor_copy(
                        dst[:, mt * L + c * 512 : mt * L + (c + 1) * 512], acc[mt][:]
                    )

        for nm, wsb, dst in (("k", wk_sb, kT_sb), ("q", wq_sb, qT_sb)):
            acc = [
                ps_proj.tile([128, 512], F32, tag="ps_proj", name=f"{nm}{c}_{mt}")
                for mt in range(2)
            ]
            for e in range(EC):
                yield lambda nm=nm, wsb=wsb, dst=dst, e=e, acc=acc: kq_mms(nm, wsb, dst, e, acc)

    def emit_proj_v(c):
        def v_mms(st4, eh, box):
            st = c * 4 + st4
            if eh == 0:
                box["vacc"] = ps_proj.tile([128, DH], F32, tag="ps_proj", name=f"vacc{st}")
            for e in range(eh * 4, eh * 4 + 4):
                nc.tensor.matmul(
                    box["vacc"][:],
                    lhsT=xtiles[("v", c)][:, e * 512 + st4 * 128 : e * 512 + (st4 + 1) * 128],
                    rhs=wv_sb[:, e * DH : (e + 1) * DH],
                    start=(e == 0),
                    stop=(e == EC - 1),
                )
            if eh == 1:
                nc.vector.tensor_copy(v_sb[:, st * DH : (st + 1) * DH], box["vacc"][:])

        for st4 in range(4):
            box = {}
            for eh in range(2):
                yield lambda st4=st4, eh=eh, box=box: v_mms(st4, eh, box)

    out3 = out.rearrange("(b p) l -> p b l", p=128)

    def emit_outproj(lc):
        for ebp in range(4):
            def unit(ebp=ebp):
                ot = ou_pool.tile([128, E], BF16, tag="ou", name=f"ot{lc}_{ebp}")
                for sub in range(2):
                    eb = ebp * 2 + sub
                    ps = ps_proj.tile(
                        [128, 512], F32, tag="ps_proj", name=f"ops{lc}_{eb}"
                    )
                    for m in range(2):
                        nc.tensor.matmul(
                            ps[:],
                            lhsT=wo_sb[:, m * E + eb * 128 : m * E + (eb + 1) * 128],
                            rhs=oT_sb[:, m * L + lc * 512 : m * L + (lc + 1) * 512],
                            start=(m == 0),
                            stop=(m == 1),
                        )
                    nc.vector.tensor_copy(ot[:, sub * 512 : (sub + 1) * 512], ps[:])
                if lc == LC - 1:
                    # tail: halve per-queue transfer time by splitting
                    # each store across both DMA queues
                    for sub in range(2):
                        eb = ebp * 2 + sub
                        eng = (nc.sync, nc.gpsimd)[sub]
                        eng.dma_start(
                            out[eb * 128 : (eb + 1) * 128, lc * 512 : (lc + 1) * 512],
                            ot[:, sub * 512 : (sub + 1) * 512],
                        )
                else:
                    eng = (nc.sync, nc.gpsimd)[(lc * 4 + ebp) % 2]
                    eng.dma_start(
                        out3[:, ebp * 2 : (ebp + 1) * 2, lc * 512 : (lc + 1) * 512],
                        ot[:].rearrange("p (b l) -> p b l", l=512),
                    )
            yield unit

    # ---- main pipeline ------------------------------------------------
    # Attention is software-pipelined: the scores matmuls for step n+1
    # are emitted BEFORE the attention-output matmuls of step n, so the
    # in-order PE never sits between two consecutive ACT activations on
    # the critical path.  ACT then runs back-to-back (~2us/step), which
    # is the steady-state limit for lc >= 1.
    def scores_step(lc, st):
        sc_tiles = {}
        for pair in range(2):
            ps = ps_sc.tile([128, 1024], F32, tag="ps_sc", name=f"scps{lc}_{st}_{pair}")
            for sub in range(2):
                nc.tensor.matmul(
                    ps[:, sub * 512 : (sub + 1) * 512],
                    lhsT=kT_sb[
                        sub * 64 : (sub + 1) * 64,
                        pair * L + st * 128 : pair * L + (st + 1) * 128,
                    ],
                    rhs=qT_sb[
                        sub * 64 : (sub + 1) * 64,
                        pair * L + lc * 512 : pair * L + (lc + 1) * 512,
                    ],
                    start=True,
                    stop=True,
                    tile_position=(sub * 64, 0),
                )
            sc = sc_pool.tile([128, 1024], BF16, tag="sc", name=f"sc{lc}_{st}_{pair}")
            nc.scalar.activation(sc[:], ps[:], AF.Sigmoid, scale=SCALE)
            sc_tiles[pair] = sc
        return sc_tiles

    def attnout_step(lc, st, sc_tiles, o_acc):
        for pair in range(2):
            for sub in range(2):
                h = pair * 2 + sub
                nc.tensor.matmul(
                    o_acc[pair][sub * 64 : (sub + 1) * 64, :],
                    lhsT=v_sb[:, st * DH + h * 64 : st * DH + (h + 1) * 64],
                    rhs=sc_tiles[pair][:, sub * 512 : (sub + 1) * 512],
                    start=(st == 0),
                    stop=(st == ST - 1),
                    tile_position=(0, sub * 64),
                    # Sim's psum-group bookkeeping mis-addresses
                    # partition-offset groups; has_written is per-element
                    # on HW and the two halves are disjoint.
                    skip_group_check=True,
                )

    filler = []          # queue of pending closures

    def pop_filler(n):
        for _ in range(min(n, len(filler))):
            filler.pop(0)()

    def flush_filler():
        while filler:
            filler.pop(0)()

    # Warm the PE (HAM clock gate) with scratch matmuls while the first
    # DMAs are in flight; they have no data dependencies.  Also preload
    # the sigmoid ACT table so the first real activation skips the
    # ~1.3us table load.
    scratch = persist.tile([128, 512], BF16, tag="scratch", name="scratch")
    nc.gpsimd.memset(scratch[:], 0.0)
    warm_in = persist.tile([128, 8], BF16, tag="warm_in", name="warm_in")
    warm_out = persist.tile([128, 8], BF16, tag="warm_out", name="warm_out")
    nc.scalar.memzero(warm_in[:])
    nc.scalar.activation(warm_out[:], warm_in[:], AF.Sigmoid, scale=SCALE)
    wu_ps = ps_sc.tile([128, 1024], F32, tag="ps_sc", name="warmup_ps")
    for i in range(10):
        nc.tensor.matmul(
            wu_ps[:, :512], lhsT=scratch[:, :128], rhs=scratch[:],
            start=(i == 0), stop=(i == 9),
        )

    # stage all inputs; run chunk 0's projections now so the first
    # scores matmuls reach the PE as early as possible
    dma_startup()
    for u in emit_proj_kq(0):
        u()
    for u in emit_proj_v(0):
        u()

    steps = [(lc, st) for lc in range(LC) for st in range(ST)]
    o_cur = None
    pend_sc = None
    for i, (lc, st) in enumerate(steps):
        if lc == 0 and st % 4 == 0 and st < 12:
            filler.extend(emit_proj_kq(st // 4 + 1))
            filler.extend(emit_proj_v(st // 4 + 1))
        if st == 0:
            o_cur = [
                ps_o.tile([128, 512], F32, tag="ps_o", name=f"oacc{lc}_{p}")
                for p in range(2)
            ]
        if pend_sc is None:
            pend_sc = scores_step(lc, st)
        # in lc0 the next sg's scores need that chunk's kT, which the
        # filler below is still producing -- don't look ahead across the
        # sg barrier there
        sg_barrier = lc == 0 and st % 4 == 3
        lookahead = None
        if i + 1 < len(steps) and not sg_barrier:
            lookahead = scores_step(*steps[i + 1])
        attnout_step(lc, st, pend_sc, o_cur)
        pend_sc = lookahead
        # filler after: the first steps' scores must reach the PE early
        # (ramp); later steps stall on ACT slots anyway
        pop_filler(11 if lc == 0 else 1)
        if sg_barrier:
            flush_filler()
        if st == ST - 1:
            for pair in range(2):
                nc.vector.tensor_copy(
                    oT_sb[:, pair * L + lc * 512 : pair * L + (lc + 1) * 512],
                    o_cur[pair][:],
                )
            filler.extend(emit_outproj(lc))
            if lc == LC - 1:
                flush_filler()

    # release pools in reverse allocation (stack) order
    for pool in (ps_o, ps_sc, ps_proj, xpool, ou_pool, sc_pool, persist):
        pool.release()


_NC_CACHE = None


def _get_nc():
    global _NC_CACHE
    if _NC_CACHE is None:
        _NC_CACHE = build_nc()
    return _NC_CACHE


def _prep_in_maps(query, key, value, Wq, Wk, Wv, Wo):
    B = query.shape[0]
    xT = {}
    for b in range(B):
        xT[("q", b)] = np.ascontiguousarray(query[b].T).astype(ml_dtypes.bfloat16)
        xT[("k", b)] = np.ascontiguousarray(key[b].T).astype(ml_dtypes.bfloat16)
        xT[("v", b)] = np.ascontiguousarray(value[b].T).astype(ml_dtypes.bfloat16)
    in_maps = []
    for c in range(N_CORES):
        b, g = c // 4, c % 4
        hs = slice(g * DH, (g + 1) * DH)
        in_maps.append(
            {
                "xqT": xT[("q", b)],
                "xkT": xT[("k", b)],
                "xvT": xT[("v", b)],
                "wq": np.ascontiguousarray(Wq[hs, :].T).astype(ml_dtypes.bfloat16),
                "wk": np.ascontiguousarray(Wk[hs, :].T).astype(ml_dtypes.bfloat16),
                "wv": np.ascontiguousarray(Wv[hs, :].T).astype(ml_dtypes.bfloat16),
                "wo": np.ascontiguousarray(Wo[:, hs].T).astype(ml_dtypes.bfloat16),
            }
        )
    return in_maps


LAST_RESULTS = None


def run_sharded(query, key, value, Wq, Wk, Wv, Wo, trace=False, tmpdir=None):
    global LAST_RESULTS
    if trace:
        # Shim the missing antenv.axon_hooks so NTFF tracing works under axon.
        import sys
        import types

        try:
            import antenv.axon_hooks  # noqa: F401
        except ImportError:
            from trn_agent_boot.trn_boot import _ntff_profile_via_ctypes

            _mod = types.ModuleType("antenv.axon_hooks")
            _hook = _ntff_profile_via_ctypes("/opt/axon/libaxon_pjrt.so")
            _mod.get_axon_ntff_profile_hook = lambda: _hook
            sys.modules["antenv.axon_hooks"] = _mod
        bass_utils.upload_artifacts = lambda tmpdir: tmpdir

    nc = _get_nc()
    in_maps = _prep_in_maps(query, key, value, Wq, Wk, Wv, Wo)
    res = bass_utils.run_bass_kernel_spmd(
        nc, in_maps, core_ids=list(range(N_CORES)), trace=trace, tmpdir=tmpdir
    )
    LAST_RESULTS = res
    B = query.shape[0]
    full = np.zeros((B, L, E), dtype=np.float32)
    for c in range(N_CORES):
        full[c // 4] += res.results[c]["out"].astype(np.float32).T
    return full


def kernel(query, key, value, Wq, Wk, Wv, Wo):
    return run_sharded(query, key, value, Wq, Wk, Wv, Wo, trace=False)


# revision 23
# speedup vs baseline: 1.0624x; 1.0313x over previous
"""Chunked sigmoid MHA on 8 Trainium2 NeuronCores (Bass/Tile).

Problem: out = (sigmoid(scale * (x_q Wq^T)(x_k Wk^T)^T) @ (x_v Wv^T)) @ Wo^T
with B=2, L=S=2048, E=1024, H=16, D=64.

Sharding: (batch, head-group) — core c handles batch b=c//4 and heads
[4g, 4g+4) with g=c%4.  Each core computes its 4 heads' Q/K/V projections
(column slices of Wq/Wk/Wv), full sigmoid attention for those heads, and a
partial output projection (row slice of Wo^T); the host sums the 4 partial
outputs per batch.

Device layouts (per core):
  xqT/xkT/xvT [E=1024, L=2048]   host-transposed activations
  wq/wk/wv    [E=1024, 256]      (Wq[g*256:(g+1)*256, :]).T
  wo          [256, E=1024]      (Wo[:, g*256:(g+1)*256]).T
  out         [L=2048, E=1024]   bf16 partial output (host sums in f32)

Input staging: one coalesced dma_start per (tensor, L-chunk) with a
[128, 8, 512] strided access pattern (all 8 E-chunks at once), spread
across four engine DMA queues (sync=xk, scalar=xq, vector=xv,
gpsimd=weights).  Per-dma_start issue overhead (~0.6us) and queue
serialization made the old 100-small-DMA staging take ~55us; this lands
chunk 0 by ~10us and everything by ~40us, which the just-in-time
projection filler schedule tolerates.

All matmuls run bf16.  Scores matmuls are row-tiled (K=64: two heads
packed in array rows 0-63 / 64-127); attention-output matmuls are
col-tiled (M=64: two heads packed in array cols 0-63 / 64-127 -> PSUM
partition halves).
"""

import ml_dtypes
import numpy as np

import concourse.bass as bass
import concourse.mybir as mybir
import concourse.tile as tile
from concourse import bass_utils
from concourse.vector_clock import ScopedClock

F32 = mybir.dt.float32
F32R = mybir.dt.float32r
BF16 = mybir.dt.bfloat16
AF = mybir.ActivationFunctionType

E = 1024          # embed dim
L = 2048          # sequence length (queries == keys)
DH = 256          # per-core projection dim (4 heads x 64)
EC = E // 128     # 8 E-chunks of 128
LC = L // 512     # 4 L-chunks of 512
ST = L // 128     # 16 S-tiles of 128
SCALE = 64 ** -0.5  # 0.125, applied inside the sigmoid activation

N_CORES = 8


class SplitDrainTileContext(tile.TileContext):
    """This walrus build rejects >1 sync wait on the SP CTRL (Drain)
    instruction, and Tile's end-of-kernel drain waits on every used proc.
    Split the waits across a chain of single-wait drains."""

    DRAIN_WAIT_CAP = 1

    def _drain_and_barrier(self, tick_clock, wait_clock):
        nc = self.nc
        drain_inst = nc.sync.drain()
        wait_clock.add_sem_waits(
            drain_inst.ins, ScopedClock({None: tick_clock.global_clock})
        )
        si = drain_inst.ins.sync_info
        waits = list(si.on_wait) if si is not None else []
        if len(waits) > self.DRAIN_WAIT_CAP:
            si.on_wait = waits[: self.DRAIN_WAIT_CAP]
            for i in range(self.DRAIN_WAIT_CAP, len(waits), self.DRAIN_WAIT_CAP):
                extra = nc.sync.drain()
                esi = extra.ins.sync_info
                if esi is None:
                    esi = mybir.SyncInfo(on_wait=[], on_update=[])
                esi.on_wait = waits[i : i + self.DRAIN_WAIT_CAP]
                extra.ins.sync_info = esi
        nc.all_engine_barrier()
        assert self.sems is not None
        popped = nc._tile_sem_poison_stack.pop()
        assert popped is self._sem_poison
        nc.clear_and_free_semaphores(list(self.sems.allocated().values()))
        nc.all_engine_barrier()


def build_nc() -> bass.Bass:
    nc = bass.Bass("TRN2", target_bir_lowering=False, debug=False)

    xq = nc.dram_tensor("xqT", [E, L], BF16, kind="ExternalInput").ap()
    xk = nc.dram_tensor("xkT", [E, L], BF16, kind="ExternalInput").ap()
    xv = nc.dram_tensor("xvT", [E, L], BF16, kind="ExternalInput").ap()
    wq = nc.dram_tensor("wq", [E, DH], BF16, kind="ExternalInput").ap()
    wk = nc.dram_tensor("wk", [E, DH], BF16, kind="ExternalInput").ap()
    wv = nc.dram_tensor("wv", [E, DH], BF16, kind="ExternalInput").ap()
    wo = nc.dram_tensor("wo", [DH, E], BF16, kind="ExternalInput").ap()
    # transposed output [E, L]: the output projection runs with the
    # (early-loaded) wo as the PE stationary operand and the
    # freshly-computed oT as the moving operand, avoiding LDWEIGHTS
    # stalls on the just-cast oT tiles.  The host transposes back.
    out = nc.dram_tensor("out", [E, L], BF16, kind="ExternalOutput").ap()

    with SplitDrainTileContext(nc) as tc:
        body(tc, xq, xk, xv, wq, wk, wv, wo, out)
    _split_waits(nc)
    return nc


def _split_waits(nc, cap=1):
    """This walrus build rejects instructions carrying more than one sync
    wait.  Hoist excess waits onto same-engine NoOps inserted immediately
    before the instruction (engine program order enforces them first)."""
    ctr = 0
    for f in nc.m.functions:
        for bb in f.blocks:
            new = []
            for inst in bb.instructions:
                si = inst.sync_info
                waits = list(si.on_wait) if si is not None else []
                if len(waits) > cap:
                    for i in range(cap, len(waits), cap):
                        ctr += 1
                        nop = mybir.InstNoOp(name=f"I-waitnop-{ctr}")
                        nop.engine = inst.engine
                        nop.sync_info = mybir.SyncInfo(
                            on_wait=waits[i : i + cap], on_update=[]
                        )
                        nc.register_instruction(nop)
                        new.append(nop)
                    si.on_wait = waits[:cap]
                new.append(inst)
            bb.instructions = new
    return ctr


def body(tc, xq, xk, xv, wq, wk, wv, wo, out):
    nc = tc.nc

    # ---- persistent SBUF tensors -------------------------------------
    persist = tc.alloc_tile_pool(name="persist", bufs=1)

    def ptile(name, shape):
        return persist.tile(shape, BF16, tag=name, name=name)

    # weights, E-chunk-major: w*_sb[:, e*256+m] = w*T[e*128+p, m]
    wq_sb = ptile("wq_sb", [128, EC * DH])
    wk_sb = ptile("wk_sb", [128, EC * DH])
    wv_sb = ptile("wv_sb", [128, EC * DH])
    # wo, m-chunk-major: wo_sb[:, m*1024+e] = wo[m*128+p, e]
    wo_sb = ptile("wo_sb", [128, 2 * E])
    # projected tensors: qT/kT [dh, L] stored Mt-major; v natural [S, dh]
    # stored St-major; oT [dh, L] stored m-chunk-major
    qT_sb = ptile("qT_sb", [128, 2 * L])
    kT_sb = ptile("kT_sb", [128, 2 * L])
    v_sb = persist.tile([128, ST * DH], BF16, tag="v_sb", name="v_sb")
    oT_sb = ptile("oT_sb", [128, 2 * L])

    sc_pool = tc.alloc_tile_pool(name="sc", bufs=8)
    ou_pool = tc.alloc_tile_pool(name="ou", bufs=3)
    xpool = tc.alloc_tile_pool(name="xpool", bufs=3 * LC)
    ps_proj = tc.alloc_tile_pool(name="ps_proj", bufs=2, space="PSUM")
    ps_sc = tc.alloc_tile_pool(name="ps_sc", bufs=2, space="PSUM")
    ps_o = tc.alloc_tile_pool(name="ps_o", bufs=2, space="PSUM")

    # ---- coalesced input staging -------------------------------------
    # x tiles: one [128, EC*512] tile per (tensor, L-chunk); the DMA
    # gathers all 8 E-chunks in one strided dma_start.  Queue assignment:
    # sync=xk, scalar=xq, vector=xv (each also carries its weight first),
    # gpsimd=wo (SWDGE, needed late).  Chunk 0 of k/q is split in two
    # dma_starts so the first projection matmuls can start earlier.
    xtiles = {}

    def dma_startup():
        w3 = {n: w.rearrange("(e p) m -> p e m", p=128) for n, w in
              (("k", wk), ("q", wq), ("v", wv))}
        x3 = {n: x.rearrange("(e p) l -> p e l", p=128) for n, x in
              (("k", xk), ("q", xq), ("v", xv))}

        def xtile(nm, c):
            xt = xpool.tile([128, EC * 512], BF16, tag="x", name=f"x{nm}{c}")
            xtiles[(nm, c)] = xt
            return (xt[:].rearrange("p (e l) -> p e l", l=512),
                    x3[nm][:, :, c * 512 : (c + 1) * 512])

        # Strict priority on the two fast HWDGE queues: critical chunk-0
        # bytes + weights first, later chunks behind them.  The 16 DMA
        # engines are shared, so any concurrently-queued non-critical
        # transfer steals bandwidth from the startup-critical prefix.
        # gpsimd (slow SWDGE) carries only wo, which is needed last.
        nc.sync.dma_start(
            wk_sb[:].rearrange("p (e m) -> p e m", m=DH), w3["k"])
        dst, src = xtile("k", 0)
        nc.sync.dma_start(dst[:, 0:4, :], src[:, 0:4, :])
        nc.sync.dma_start(dst[:, 4:8, :], src[:, 4:8, :])
        dst, src = xtile("v", 0)
        nc.sync.dma_start(dst[:, 0:4, :], src[:, 0:4, :])
        nc.sync.dma_start(dst[:, 4:8, :], src[:, 4:8, :])
        nc.scalar.dma_start(
            wq_sb[:].rearrange("p (e m) -> p e m", m=DH), w3["q"])
        dst, src = xtile("q", 0)
        nc.scalar.dma_start(dst[:, 0:4, :], src[:, 0:4, :])
        nc.scalar.dma_start(dst[:, 4:8, :], src[:, 4:8, :])
        nc.scalar.dma_start(
            wv_sb[:].rearrange("p (e m) -> p e m", m=DH), w3["v"])
        nc.sync.dma_start(*xtile("k", 1))
        nc.scalar.dma_start(*xtile("q", 1))
        nc.scalar.dma_start(*xtile("v", 1))
        nc.sync.dma_start(*xtile("k", 2))
        nc.scalar.dma_start(*xtile("q", 2))
        nc.sync.dma_start(*xtile("k", 3))
        nc.scalar.dma_start(*xtile("q", 3))
        nc.sync.dma_start(*xtile("v", 2))
        nc.scalar.dma_start(*xtile("v", 3))
        nc.gpsimd.dma_start(
            wo_sb[:].rearrange("p (m e) -> p m e", e=E),
            wo.rearrange("(m p) e -> p m e", p=128))

    def emit_proj_kq(c, tensors=("k", "q")):
        """Yield closures emitting one L-chunk of the k/q projection
        matmuls (x tiles must already be staged via dma_startup).

        NOTE: the 8 units per tensor must stay CONTIGUOUS in the filler
        queue: their PSUM accumulators stay live across all 8, and any
        interleaved ps_proj allocation would rotate onto a live buffer
        and deadlock the in-order PE."""

        def kq_mms(nm, wsb, dst, e, acc):
            for mt in range(2):
                nc.tensor.matmul(
                    acc[mt][:],
                    lhsT=wsb[:, e * DH + mt * 128 : e * DH + (mt + 1) * 128],
                    rhs=xtiles[(nm, c)][:, e * 512 : (e + 1) * 512],
                    start=(e == 0),
                    stop=(e == EC - 1),
                )
            if e == EC - 1:
                for mt in range(2):
                    nc.vector.tensor_copy(
                        dst[:, mt * L + c * 512 : mt * L + (c + 1) * 512], acc[mt][:]
                    )

        for nm, wsb, dst in (("k", wk_sb, kT_sb), ("q", wq_sb, qT_sb)):
            if nm not in tensors:
                continue
            acc = [
                ps_proj.tile([128, 512], F32, tag="ps_proj", name=f"{nm}{c}_{mt}")
                for mt in range(2)
            ]
            for e in range(EC):
                yield lambda nm=nm, wsb=wsb, dst=dst, e=e, acc=acc: kq_mms(nm, wsb, dst, e, acc)

    def emit_proj_v(c):
        def v_mms(st4, eh, box):
            st = c * 4 + st4
            if eh == 0:
                box["vacc"] = ps_proj.tile([128, DH], F32, tag="ps_proj", name=f"vacc{st}")
            for e in range(eh * 4, eh * 4 + 4):
                nc.tensor.matmul(
                    box["vacc"][:],
                    lhsT=xtiles[("v", c)][:, e * 512 + st4 * 128 : e * 512 + (st4 + 1) * 128],
                    rhs=wv_sb[:, e * DH : (e + 1) * DH],
                    start=(e == 0),
                    stop=(e == EC - 1),
                )
            if eh == 1:
                nc.vector.tensor_copy(v_sb[:, st * DH : (st + 1) * DH], box["vacc"][:])

        for st4 in range(4):
            box = {}
            for eh in range(2):
                yield lambda st4=st4, eh=eh, box=box: v_mms(st4, eh, box)

    out3 = out.rearrange("(b p) l -> p b l", p=128)

    def emit_outproj(lc):
        for ebp in range(4):
            def unit(ebp=ebp):
                ot = ou_pool.tile([128, E], BF16, tag="ou", name=f"ot{lc}_{ebp}")
                for sub in range(2):
                    eb = ebp * 2 + sub
                    ps = ps_proj.tile(
                        [128, 512], F32, tag="ps_proj", name=f"ops{lc}_{eb}"
                    )
                    for m in range(2):
                        nc.tensor.matmul(
                            ps[:],
                            lhsT=wo_sb[:, m * E + eb * 128 : m * E + (eb + 1) * 128],
                            rhs=oT_sb[:, m * L + lc * 512 : m * L + (lc + 1) * 512],
                            start=(m == 0),
                            stop=(m == 1),
                        )
                    nc.vector.tensor_copy(ot[:, sub * 512 : (sub + 1) * 512], ps[:])
                if lc == LC - 1:
                    # tail: halve per-queue transfer time by splitting
                    # each store across both DMA queues
                    for sub in range(2):
                        eb = ebp * 2 + sub
                        eng = (nc.sync, nc.gpsimd)[sub]
                        eng.dma_start(
                            out[eb * 128 : (eb + 1) * 128, lc * 512 : (lc + 1) * 512],
                            ot[:, sub * 512 : (sub + 1) * 512],
                        )
                else:
                    eng = (nc.sync, nc.gpsimd)[(lc * 4 + ebp) % 2]
                    eng.dma_start(
                        out3[:, ebp * 2 : (ebp + 1) * 2, lc * 512 : (lc + 1) * 512],
                        ot[:].rearrange("p (b l) -> p b l", l=512),
                    )
            yield unit

    # ---- main pipeline ------------------------------------------------
    # Attention is software-pipelined: the scores matmuls for step n+1
    # are emitted BEFORE the attention-output matmuls of step n, so the
    # in-order PE never sits between two consecutive ACT activations on
    # the critical path.  ACT then runs back-to-back (~2us/step), which
    # is the steady-state limit for lc >= 1.
    def scores_step(lc, st):
        sc_tiles = {}
        for pair in range(2):
            ps = ps_sc.tile([128, 1024], F32, tag="ps_sc", name=f"scps{lc}_{st}_{pair}")
            for sub in range(2):
                nc.tensor.matmul(
                    ps[:, sub * 512 : (sub + 1) * 512],
                    lhsT=kT_sb[
                        sub * 64 : (sub + 1) * 64,
                        pair * L + st * 128 : pair * L + (st + 1) * 128,
                    ],
                    rhs=qT_sb[
                        sub * 64 : (sub + 1) * 64,
                        pair * L + lc * 512 : pair * L + (lc + 1) * 512,
                    ],
                    start=True,
                    stop=True,
                    tile_position=(sub * 64, 0),
                )
            sc = sc_pool.tile([128, 1024], BF16, tag="sc", name=f"sc{lc}_{st}_{pair}")
            nc.scalar.activation(sc[:], ps[:], AF.Sigmoid, scale=SCALE)
            sc_tiles[pair] = sc
        return sc_tiles

    def attnout_step(lc, st, sc_tiles, o_acc):
        for pair in range(2):
            for sub in range(2):
                h = pair * 2 + sub
                nc.tensor.matmul(
                    o_acc[pair][sub * 64 : (sub + 1) * 64, :],
                    lhsT=v_sb[:, st * DH + h * 64 : st * DH + (h + 1) * 64],
                    rhs=sc_tiles[pair][:, sub * 512 : (sub + 1) * 512],
                    start=(st == 0),
                    stop=(st == ST - 1),
                    tile_position=(0, sub * 64),
                    # Sim's psum-group bookkeeping mis-addresses
                    # partition-offset groups; has_written is per-element
                    # on HW and the two halves are disjoint.
                    skip_group_check=True,
                )

    filler = []          # queue of pending closures

    def pop_filler(n):
        for _ in range(min(n, len(filler))):
            filler.pop(0)()

    def flush_filler():
        while filler:
            filler.pop(0)()

    # Warm the PE (HAM clock gate) with scratch matmuls while the first
    # DMAs are in flight; they have no data dependencies.  Also preload
    # the sigmoid ACT table so the first real activation skips the
    # ~1.3us table load.
    scratch = persist.tile([128, 512], BF16, tag="scratch", name="scratch")
    nc.gpsimd.memset(scratch[:], 0.0)
    warm_in = persist.tile([128, 8], BF16, tag="warm_in", name="warm_in")
    warm_out = persist.tile([128, 8], BF16, tag="warm_out", name="warm_out")
    nc.scalar.memzero(warm_in[:])
    nc.scalar.activation(warm_out[:], warm_in[:], AF.Sigmoid, scale=SCALE)
    wu_ps = ps_sc.tile([128, 1024], F32, tag="ps_sc", name="warmup_ps")
    for i in range(10):
        nc.tensor.matmul(
            wu_ps[:, :512], lhsT=scratch[:, :128], rhs=scratch[:],
            start=(i == 0), stop=(i == 9),
        )

    # stage all inputs; run chunk 0's projections now so the first
    # scores matmuls reach the PE as early as possible
    dma_startup()
    for u in emit_proj_kq(0):
        u()
    for u in emit_proj_v(0):
        u()

    steps = [(lc, st) for lc in range(LC) for st in range(ST)]
    o_cur = None
    pend_sc = None
    for i, (lc, st) in enumerate(steps):
        if lc == 0 and st % 4 == 0 and st < 12:
            # lc0 only needs kT/v of every chunk and qT of chunk 1 (for
            # the lc1 lookahead); defer q projections for chunks 2/3
            # into lc1/lc2, whose ACT-bound steps leave the PE idle
            c = st // 4 + 1
            filler.extend(emit_proj_kq(c, ("k", "q") if c == 1 else ("k",)))
            filler.extend(emit_proj_v(c))
        if lc in (1, 2) and st == 0:
            # behind outproj(lc-1) in the queue; at 1 unit/step these 8
            # units finish by step 12, before the (lc+1, 0) lookahead
            filler.extend(emit_proj_kq(lc + 1, ("q",)))
        if st == 0:
            o_cur = [
                ps_o.tile([128, 512], F32, tag="ps_o", name=f"oacc{lc}_{p}")
                for p in range(2)
            ]
        if pend_sc is None:
            pend_sc = scores_step(lc, st)
        # in lc0 the next sg's scores need that chunk's kT, which the
        # filler below is still producing -- don't look ahead across the
        # sg barrier there
        sg_barrier = lc == 0 and st % 4 == 3
        lookahead = None
        if i + 1 < len(steps) and not sg_barrier:
            lookahead = scores_step(*steps[i + 1])
        attnout_step(lc, st, pend_sc, o_cur)
        pend_sc = lookahead
        # filler after: the first steps' scores must reach the PE early
        # (ramp); later steps stall on ACT slots anyway
        pop_filler(11 if lc == 0 else 1)
        if sg_barrier:
            flush_filler()
        if st == ST - 1:
            for pair in range(2):
                nc.vector.tensor_copy(
                    oT_sb[:, pair * L + lc * 512 : pair * L + (lc + 1) * 512],
                    o_cur[pair][:],
                )
            filler.extend(emit_outproj(lc))
            if lc == LC - 1:
                flush_filler()

    # release pools in reverse allocation (stack) order
    for pool in (ps_o, ps_sc, ps_proj, xpool, ou_pool, sc_pool, persist):
        pool.release()


_NC_CACHE = None


def _get_nc():
    global _NC_CACHE
    if _NC_CACHE is None:
        _NC_CACHE = build_nc()
    return _NC_CACHE


def _prep_in_maps(query, key, value, Wq, Wk, Wv, Wo):
    B = query.shape[0]
    xT = {}
    for b in range(B):
        xT[("q", b)] = np.ascontiguousarray(query[b].T).astype(ml_dtypes.bfloat16)
        xT[("k", b)] = np.ascontiguousarray(key[b].T).astype(ml_dtypes.bfloat16)
        xT[("v", b)] = np.ascontiguousarray(value[b].T).astype(ml_dtypes.bfloat16)
    in_maps = []
    for c in range(N_CORES):
        b, g = c // 4, c % 4
        hs = slice(g * DH, (g + 1) * DH)
        in_maps.append(
            {
                "xqT": xT[("q", b)],
                "xkT": xT[("k", b)],
                "xvT": xT[("v", b)],
                "wq": np.ascontiguousarray(Wq[hs, :].T).astype(ml_dtypes.bfloat16),
                "wk": np.ascontiguousarray(Wk[hs, :].T).astype(ml_dtypes.bfloat16),
                "wv": np.ascontiguousarray(Wv[hs, :].T).astype(ml_dtypes.bfloat16),
                "wo": np.ascontiguousarray(Wo[:, hs].T).astype(ml_dtypes.bfloat16),
            }
        )
    return in_maps


LAST_RESULTS = None


def run_sharded(query, key, value, Wq, Wk, Wv, Wo, trace=False, tmpdir=None):
    global LAST_RESULTS
    if trace:
        # Shim the missing antenv.axon_hooks so NTFF tracing works under axon.
        import sys
        import types

        try:
            import antenv.axon_hooks  # noqa: F401
        except ImportError:
            from trn_agent_boot.trn_boot import _ntff_profile_via_ctypes

            _mod = types.ModuleType("antenv.axon_hooks")
            _hook = _ntff_profile_via_ctypes("/opt/axon/libaxon_pjrt.so")
            _mod.get_axon_ntff_profile_hook = lambda: _hook
            sys.modules["antenv.axon_hooks"] = _mod
        bass_utils.upload_artifacts = lambda tmpdir: tmpdir

    nc = _get_nc()
    in_maps = _prep_in_maps(query, key, value, Wq, Wk, Wv, Wo)
    res = bass_utils.run_bass_kernel_spmd(
        nc, in_maps, core_ids=list(range(N_CORES)), trace=trace, tmpdir=tmpdir
    )
    LAST_RESULTS = res
    B = query.shape[0]
    full = np.zeros((B, L, E), dtype=np.float32)
    for c in range(N_CORES):
        full[c // 4] += res.results[c]["out"].astype(np.float32).T
    return full


def kernel(query, key, value, Wq, Wk, Wv, Wo):
    return run_sharded(query, key, value, Wq, Wk, Wv, Wo, trace=False)
